# revision 2
# baseline (speedup 1.0000x reference)
"""Trainium2 Bass kernel: 8-layer ternary (BitNet-1.58) dense transformer.

Model (per reference):
    h = embed[input_ids]                                  # (B=2, S=1024, H=2048)
    8x: y = h @ ternary(W_l)^T + b_l ; h = LN(y + h)*g+b  # H=2048
    h = LN(h)*final_g + final_b
    logits = h @ ternary(head_W)^T                        # (B, S, V=32000)

Sharding over 8 NeuronCores:
  - Layers: data-parallel over the 2048 tokens (256 tokens/core). Each core
    streams the full layer weights; no collectives.
  - Head: 8-way tensor-parallel over vocab (4000 vocab rows/core). Final
    hidden states are exchanged with two AllGathers (one per 128-token tile)
    of fp16 transposed activations; each core computes all 2048 tokens x its
    vocab shard. Local tokens are computed from SBUF before/while the gather
    runs; remote tokens stream from the gathered buffer using the runtime
    partition id for ring addressing.

Precision: ternary weights are scaled by an exact power of two (2^-e, e~6)
so they are EXACT in fp8e4; the compensating factor (s * 2^e ~= 1) is folded
into the activation fp16 cast that feeds the transpose. Matmuls run
fp16 (activations, stationary) x fp8 (weights, moving); PSUM accumulates
fp32; the residual/LN path stays fp32. Logits are written bf16 and upcast
on the host.
"""

import os
import sys

import numpy as np

try:
    import concourse.bass as bass
except ImportError:  # grading container should have it on sys.path already
    sys.path.insert(0, "/opt/trn_rl_repo")
    import concourse.bass as bass

import ml_dtypes
import concourse.mybir as mybir
import concourse.tile as tile
from concourse import bacc
from concourse.bass import ts
from concourse.bass_utils import run_bass_kernel_spmd
from contextlib import ExitStack

F32 = mybir.dt.float32
BF16 = mybir.dt.bfloat16
FP16 = mybir.dt.float16
FP8 = mybir.dt.float8e4
AX = mybir.AxisListType
OP = mybir.AluOpType
AF = mybir.ActivationFunctionType
EPS = 1e-5

# Full-size problem config (B=2, S=1024 -> 2048 tokens).
CFG_FULL = dict(L=8, H=2048, NTOK=2048, NC=8, TT=2, VS=4000, NV=500, CH=512)


def build_nc(cfg, sigmas, head_sigma, use_gb):
    L, H, NTOK, NC, TT = cfg["L"], cfg["H"], cfg["NTOK"], cfg["NC"], cfg["TT"]
    VS, NV, CH = cfg["VS"], cfg["NV"], cfg["CH"]
    KT = H // 128
    KH = KT // 2  # k-tiles per weight half
    NCH = H // CH
    NVH = VS // 2 // NV  # vocab chunks per half (4)
    assert NTOK == NC * TT * 128 and VS % (2 * NV) == 0

    nc = bacc.Bacc("TRN2", target_bir_lowering=False, debug=False, num_devices=NC)
    h0 = nc.declare_dram_parameter("h0", [TT, 128, H], F32, isOutput=False)
    w_ = nc.declare_dram_parameter("w", [L, KT, 128, H], FP8, isOutput=False)
    if use_gb:
        lng = nc.declare_dram_parameter("lng", [L, H], BF16, isOutput=False)
        lnb = nc.declare_dram_parameter("lnb", [L, H], BF16, isOutput=False)
        lbias = nc.declare_dram_parameter("lbias", [L, H], BF16, isOutput=False)
        fing = nc.declare_dram_parameter("fing", [H], BF16, isOutput=False)
        finb = nc.declare_dram_parameter("finb", [H], BF16, isOutput=False)
    hw_ = nc.declare_dram_parameter("hw", [KT, 128, VS], FP8, isOutput=False)
    ident_d = nc.declare_dram_parameter("ident", [128, 128], FP16, isOutput=False)
    eps_d = nc.declare_dram_parameter("eps", [128, 1], F32, isOutput=False)
    out = nc.declare_dram_parameter("out", [NTOK, VS], BF16, isOutput=True)
    hT_loc = [nc.dram_tensor(f"hT_loc{t}", [128, H], FP16) for t in range(TT)]
    hT_all = [
        nc.dram_tensor(
            f"hT_all{t}",
            [NC, 128, H],
            FP16,
            addr_space="Shared" if NC > 4 else "Local",
        )
        for t in range(TT)
    ]

    pid = nc.partition_id()

    with tile.TileContext(nc) as tc:
        hT_store_insts = [None] * TT
        with ExitStack() as ctxTop:
            hwp = ctxTop.enter_context(tc.tile_pool(name="hw", bufs=1))
            hTfinp = ctxTop.enter_context(tc.tile_pool(name="hTfin", bufs=TT))
            hwt = hwp.tile([128, KT, VS], FP8)
            hTfin = []

            with ExitStack() as ctxA:
                consts = ctxA.enter_context(tc.tile_pool(name="consts", bufs=1))
                state = ctxA.enter_context(tc.tile_pool(name="state", bufs=4))
                zpool = ctxA.enter_context(tc.tile_pool(name="z", bufs=2))
                hscp = ctxA.enter_context(tc.tile_pool(name="hsc", bufs=2))
                hTp = ctxA.enter_context(tc.tile_pool(name="hT", bufs=2))
                wp = ctxA.enter_context(tc.tile_pool(name="w", bufs=3))
                gbp = None
                if use_gb:
                    gbp = ctxA.enter_context(tc.tile_pool(name="gb", bufs=2))
                smp = ctxA.enter_context(tc.tile_pool(name="small", bufs=16))
                psT = ctxA.enter_context(tc.tile_pool(name="psT", bufs=1, space="PSUM"))
                psY = ctxA.enter_context(
                    tc.tile_pool(name="psY", bufs=NCH + 2, space="PSUM")
                )

                ident = consts.tile([128, 128], FP16)
                nc.sync.dma_start(ident[:], ident_d[:])
                eps_t = consts.tile([128, 1], F32)
                nc.sync.dma_start(eps_t[:], eps_d[:])

                h_cur = []
                for t in range(TT):
                    st = state.tile([128, H], F32, name=f"hinit{t}", tag="state")
                    nc.sync.dma_start(st[:], h0[t])
                    h_cur.append(st)

                def transpose_cast(src_f32, scale_imm, pool, name):
                    """h [128tok, H] f32 -> hT [128feat-in-blk, (kt,128tok)] fp16*scale."""
                    hsc = hscp.tile([128, H], FP16, tag="hsc", name=f"hsc{name}")
                    nc.vector.tensor_scalar_mul(hsc[:], src_f32[:], float(scale_imm))
                    pT = psT.tile([128, H], FP16, tag="psT", name=f"pT{name}")
                    for kt in range(KT):
                        nc.tensor.transpose(
                            pT[:, kt * 128 : (kt + 1) * 128],
                            hsc[:, kt * 128 : (kt + 1) * 128],
                            ident[:],
                        )
                    dst = pool.tile([128, H], FP16, tag="hT", name=f"hT{name}")
                    nc.scalar.copy(dst[:], pT[:])
                    return dst

                def ln_finish(affine_src, S_ap, SS_ap, g_t, b_t, name):
                    S = smp.tile([128, 1], F32, tag="s0", name=f"S{name}")
                    SS = smp.tile([128, 1], F32, tag="s1", name=f"SS{name}")
                    nc.vector.tensor_reduce(S[:], S_ap, axis=AX.X, op=OP.add)
                    nc.vector.tensor_reduce(SS[:], SS_ap, axis=AX.X, op=OP.add)
                    negmean = smp.tile([128, 1], F32, tag="s2", name=f"nm{name}")
                    nc.vector.tensor_scalar_mul(negmean[:], S[:], -1.0 / H)
                    msq = smp.tile([128, 1], F32, tag="s3", name=f"msq{name}")
                    nc.vector.tensor_scalar_mul(msq[:], SS[:], 1.0 / H)
                    var = smp.tile([128, 1], F32, tag="s4", name=f"var{name}")
                    nc.vector.tensor_tensor(var[:], negmean[:], negmean[:], OP.mult)
                    nc.vector.tensor_tensor(var[:], msq[:], var[:], OP.subtract)
                    std = smp.tile([128, 1], F32, tag="s5", name=f"std{name}")
                    nc.scalar.activation(std[:], var[:], AF.Sqrt, bias=eps_t[:])
                    rstd = smp.tile([128, 1], F32, tag="s6", name=f"rstd{name}")
                    nc.vector.reciprocal(rstd[:], std[:])
                    hn = state.tile([128, H], F32, tag="state", name=f"h{name}")
                    nc.vector.tensor_scalar(
                        hn[:], affine_src[:], negmean[:], rstd[:], OP.add, OP.mult
                    )
                    if g_t is not None:
                        nc.vector.tensor_tensor(hn[:], hn[:], g_t[:], OP.mult)
                        nc.vector.tensor_tensor(hn[:], hn[:], b_t[:], OP.add)
                    return hn

                for l in range(L):
                    w_half = []
                    for hf in range(2):
                        wt = wp.tile([128, KH, H], FP8, tag="w", name=f"w{l}_{hf}")
                        nc.sync.dma_start(
                            wt[:],
                            w_[l, hf * KH : (hf + 1) * KH].rearrange("k p o -> p k o"),
                        )
                        w_half.append(wt)
                    # stream head-weight chunks in the shadow of the layer loop
                    nc.sync.dma_start(
                        hwt[:, 2 * l : 2 * l + 2, :],
                        hw_[2 * l : 2 * l + 2].rearrange("k p v -> p k v"),
                    )
                    g_t = b_t = bias_t = None
                    if use_gb:
                        g_t = gbp.tile([128, H], BF16, tag="g", name=f"g{l}")
                        nc.sync.dma_start(g_t[:], lng[l][None, :].to_broadcast((128, H)))
                        b_t = gbp.tile([128, H], BF16, tag="b", name=f"b{l}")
                        nc.sync.dma_start(b_t[:], lnb[l][None, :].to_broadcast((128, H)))
                        bias_t = gbp.tile([128, H], BF16, tag="bias", name=f"bias{l}")
                        nc.sync.dma_start(
                            bias_t[:], lbias[l][None, :].to_broadcast((128, H))
                        )

                    for t in range(TT):
                        hTt = transpose_cast(h_cur[t], sigmas[l], hTp, f"{l}_{t}")
                        ps = []
                        for i in range(NCH):
                            p = psY.tile(
                                [128, CH], F32, tag="psY", name=f"ps{l}_{t}_{i}"
                            )
                            ps.append(p)
                        for kt in range(KT):
                            wt = w_half[kt // KH]
                            for i in range(NCH):
                                nc.tensor.matmul(
                                    ps[i][:],
                                    lhsT=hTt[:, kt * 128 : (kt + 1) * 128],
                                    rhs=wt[:, kt % KH, i * CH : (i + 1) * CH],
                                    start=(kt == 0),
                                    stop=(kt == KT - 1),
                                )
                        z = zpool.tile([128, H], F32, tag="z", name=f"z{l}_{t}")
                        sums = smp.tile(
                            [128, 1 + NCH], F32, tag="sums", name=f"sm{l}_{t}"
                        )
                        resid = h_cur[t]
                        if use_gb:
                            hb = zpool.tile([128, H], F32, tag="hb", name=f"hb{l}_{t}")
                            nc.vector.tensor_tensor(
                                hb[:], h_cur[t][:], bias_t[:], OP.add
                            )
                            resid = hb
                        for i in range(NCH):
                            nc.vector.tensor_add(
                                z[:, i * CH : (i + 1) * CH],
                                ps[i][:],
                                resid[:, i * CH : (i + 1) * CH],
                            )
                        nc.vector.tensor_reduce(
                            sums[:, 0:1], z[:], axis=AX.X, op=OP.add
                        )
                        for i in range(NCH):
                            nc.scalar.activation(
                                ps[i][:],
                                z[:, i * CH : (i + 1) * CH],
                                AF.Square,
                                accum_out=sums[:, 1 + i : 2 + i],
                            )
                        h_cur[t] = ln_finish(
                            z, sums[:, 0:1], sums[:, 1 : 1 + NCH], g_t, b_t,
                            f"{l}_{t}",
                        )

                # final LN + head-input transposes + stores for the gather
                fg = fb = None
                if use_gb:
                    fg = gbp.tile([128, H], BF16, tag="g", name="gfin")
                    nc.sync.dma_start(fg[:], fing[None, :].to_broadcast((128, H)))
                    fb = gbp.tile([128, H], BF16, tag="b", name="bfin")
                    nc.sync.dma_start(fb[:], finb[None, :].to_broadcast((128, H)))
                for t in range(TT):
                    h8 = h_cur[t]
                    sums = smp.tile(
                        [128, 1 + NCH], F32, tag="sums", name=f"smfin{t}"
                    )
                    nc.vector.tensor_reduce(sums[:, 0:1], h8[:], axis=AX.X, op=OP.add)
                    for i in range(NCH):
                        dump = psY.tile([128, CH], F32, tag="psY", name=f"dmp{t}_{i}")
                        nc.scalar.activation(
                            dump[:],
                            h8[:, i * CH : (i + 1) * CH],
                            AF.Square,
                            accum_out=sums[:, 1 + i : 2 + i],
                        )
                    hfin = ln_finish(
                        h8, sums[:, 0:1], sums[:, 1 : 1 + NCH], fg, fb, f"fin{t}"
                    )
                    hTt = transpose_cast(hfin, head_sigma, hTfinp, f"fin{t}")
                    hTfin.append(hTt)
                    st_i = nc.sync.dma_start(hT_loc[t][:], hTt[:])
                    hT_store_insts[t] = st_i

            # two AllGathers (one per token tile) so the first can start while
            # the second tile's final LN is still in flight
            ccs = []
            for t in range(TT):
                cc = nc.gpsimd.collective_compute(
                    "AllGather",
                    OP.bypass,
                    replica_groups=[list(range(NC))],
                    ins=[hT_loc[t][:]],
                    outs=[hT_all[t][:]],
                )
                tile.add_dep_helper(
                    cc.ins,
                    hT_store_insts[t].ins,
                    sync=True,
                    reason=f"gather{t} waits on hT store{t}",
                )
                ccs.append(cc)

            with ExitStack() as ctxB:
                peerp = ctxB.enter_context(tc.tile_pool(name="peer", bufs=2))
                outp = ctxB.enter_context(tc.tile_pool(name="outstg", bufs=2))
                psH = ctxB.enter_context(
                    tc.tile_pool(name="psH", bufs=8, space="PSUM")
                )

                def head_ct(hT_src, row_slot, name):
                    """All 16 k-tiles x 4000 vocab for one 128-token tile.

                    hT_src: SBUF tile [128, H] fp16 (feature-major).
                    row_slot: (c*TT + t) — static int or runtime ScalarValue.
                    """
                    for half in range(2):
                        pss = [
                            psH.tile(
                                [128, NV], F32, tag="psH", name=f"ph{name}_{half}_{v}"
                            )
                            for v in range(NVH)
                        ]
                        for kt in range(KT):
                            for vi in range(NVH):
                                v0 = half * (NVH * NV) + vi * NV
                                nc.tensor.matmul(
                                    pss[vi][:],
                                    lhsT=hT_src[:, kt * 128 : (kt + 1) * 128],
                                    rhs=hwt[:, kt, v0 : v0 + NV],
                                    start=(kt == 0),
                                    stop=(kt == KT - 1),
                                    skip_group_check=True,
                                )
                        o_t = outp.tile(
                            [128, NVH * NV], BF16, tag="ostg", name=f"o{name}_{half}"
                        )
                        for vi in range(NVH):
                            nc.scalar.copy(
                                o_t[:, vi * NV : (vi + 1) * NV], pss[vi][:]
                            )
                        nc.sync.dma_start(
                            out[
                                ts(row_slot, 128),
                                half * (NVH * NV) : (half + 1) * (NVH * NV),
                            ],
                            o_t[:],
                        )

                # local tokens first — overlaps the AllGathers
                for t in range(TT):
                    head_ct(hTfin[t], pid * TT + t, f"loc{t}")

                # remote tokens: ring order c = (pid + r) & (NC-1)
                for r in range(1, NC):
                    c = (pid + r) & (NC - 1)
                    for t in range(TT):
                        pt = peerp.tile(
                            [128, H], FP16, tag="peer", name=f"peer{r}_{t}"
                        )
                        ld = nc.sync.dma_start(pt[:], hT_all[t][ts(c, 1), :, :])
                        tile.add_dep_helper(
                            ld.ins,
                            ccs[t].ins,
                            sync=True,
                            reason=f"peer load r{r} t{t} waits gather{t}",
                        )
                        head_ct(pt, c * TT + t, f"r{r}_{t}")

    return nc


def _ternary(wmat):
    """Exact {-1,0,1} ternary tensor + fp32 scale, matching the reference."""
    w = np.asarray(wmat, dtype=np.float32)
    s = np.mean(np.abs(w), dtype=np.float32)
    t = np.clip(np.rint(w / (s + np.float32(1e-8))), -1.0, 1.0).astype(np.float32)
    return t, float(s)


def _split_scale(s):
    """s = sigma * 2^-e with sigma ~ 1 and 2^-e exact in fp8e4."""
    e = int(np.clip(np.round(-np.log2(s)), -7, 9))
    return s * (2.0**e), e


_NC_CACHE = {}
_LAST_RESULTS = None


def kernel(**inputs):
    global _LAST_RESULTS
    cfg = CFG_FULL
    L, H, NTOK, NC, TT, VS = (
        cfg["L"], cfg["H"], cfg["NTOK"], cfg["NC"], cfg["TT"], cfg["VS"],
    )
    KT = H // 128
    TPC = TT * 128  # tokens per core
    BF = ml_dtypes.bfloat16
    F8 = ml_dtypes.float8_e4m3fn

    ids = np.asarray(inputs["input_ids"]).astype(np.int64).reshape(-1)
    embed = np.asarray(inputs["embed"], dtype=np.float32)
    layer_w = np.asarray(inputs["layer_w"], dtype=np.float32)
    layer_b = np.asarray(inputs["layer_b"], dtype=np.float32)
    ln_g = np.asarray(inputs["ln_g"], dtype=np.float32)
    ln_b = np.asarray(inputs["ln_b"], dtype=np.float32)
    final_g = np.asarray(inputs["final_g"], dtype=np.float32)
    final_b = np.asarray(inputs["final_b"], dtype=np.float32)
    head_w = np.asarray(inputs["head_w"], dtype=np.float32)

    use_gb = not (
        np.all(layer_b == 0.0)
        and np.all(ln_g == 1.0)
        and np.all(ln_b == 0.0)
        and np.all(final_g == 1.0)
        and np.all(final_b == 0.0)
    )

    h0_full = embed[ids]  # [NTOK, H] fp32

    sigmas = []
    wT = np.empty([L, KT, 128, H], dtype=F8)
    for l in range(L):
        t, s = _ternary(layer_w[l])
        sig, e = _split_scale(s)
        sigmas.append(sig)
        wT[l] = (np.ascontiguousarray(t.T) * np.float32(2.0**-e)).reshape(
            KT, 128, H
        ).astype(F8)
    th, head_scale = _ternary(head_w)
    head_sigma, e_h = _split_scale(head_scale)
    headT = (np.ascontiguousarray(th.T) * np.float32(2.0**-e_h)).astype(F8)  # [H, V]

    key = (id(cfg), tuple(sigmas), head_sigma, use_gb)
    if key not in _NC_CACHE:
        _NC_CACHE.clear()
        nc = build_nc(cfg, sigmas, head_sigma, use_gb)
        # Bacc.finalize runs the TRN2 legalization passes (1-wait-per-
        # instruction event-semaphore split, matmul->ldweights wait motion,
        # register allocation). The PJRT exec path serializes nc as-is.
        nc.finalize()
        _NC_CACHE[key] = nc
    nc = _NC_CACHE[key]

    common = {
        "w": wT,
        "ident": np.eye(128, dtype=np.float16),
        "eps": np.full((128, 1), EPS, np.float32),
    }
    if use_gb:
        common.update(
            lng=ln_g.astype(BF),
            lnb=ln_b.astype(BF),
            lbias=layer_b.astype(BF),
            fing=final_g.astype(BF),
            finb=final_b.astype(BF),
        )
    in_maps = []
    for c in range(NC):
        in_maps.append(
            dict(
                common,
                h0=np.ascontiguousarray(
                    h0_full[c * TPC : (c + 1) * TPC].reshape(TT, 128, H)
                ),
                hw=np.ascontiguousarray(
                    headT[:, c * VS : (c + 1) * VS].reshape(KT, 128, VS)
                ),
            )
        )

    trace = bool(int(os.environ.get("TRIKERNEL_TRACE", "0")))
    res = run_bass_kernel_spmd(nc, in_maps, core_ids=list(range(NC)), trace=trace)
    _LAST_RESULTS = res

    full = np.concatenate(
        [np.asarray(res.results[c]["out"]) for c in range(NC)], axis=1
    )  # [NTOK, V] bf16
    return full.reshape(2, 1024, 32000).astype(np.float32)
